# revision 35
# baseline (speedup 1.0000x reference)
"""GAT message-passing kernel for Trainium2, 8 NeuronCores (SPMD).

Two SPMD launches:
1. "table": each core computes G4.T = [W@att_src, W@att_dst, W].T @ x.T for
   its 1/8 stripe of nodes (Wcat stationary on the PE, x.T streamed).
   The host concatenates the stripes into the [N,4] node table and performs
   the pure-index gather of per-edge (a_src, h0, h1) into the slab layout
   (data movement only; all math stays on device).
2. "edges": per core, its edge slab (counting-sorted by dst, degree-grouped
   padded CSR on 128 partitions, uniform group width per chunk) streams
   through fused scalar_tensor_tensor / Prelu / Exp; segment softmax and
   the weighted aggregation are strided free-dim reduces; per-node values
   broadcast via stride-0 access patterns.

Each core owns a contiguous dst-node range (~E/8 edges), so segment
reductions are fully core-local; no collectives are needed.
"""

import os
import numpy as np

N = 100000
E = 6400000
F_IN = 128
C = 2
NC = 8
P = 128
NEG_SLOPE = 0.2
NSTR = N // NC          # 12500 nodes per x-stripe
TILES = (NSTR + P - 1) // P
NSTR_PAD = TILES * P    # 12544
KILL = -1.0e5
CHUNK = 512


def _prep(edge_index):
    src = np.ascontiguousarray(edge_index[0]).astype(np.int64)
    dst = np.ascontiguousarray(edge_index[1]).astype(np.int64)

    perm = np.argsort(dst, kind="stable")
    dst_s = dst[perm]
    src_s = src[perm]

    deg = np.bincount(dst, minlength=N).astype(np.int64)
    starts = np.zeros(N + 1, np.int64)
    np.cumsum(deg, out=starts[1:])

    targets = (np.arange(1, NC) * E) // NC
    nb = np.searchsorted(starts, targets).astype(np.int64)
    node_bounds = np.concatenate([[0], nb, [N]])

    cores = []
    max_nodes = int((node_bounds[1:] - node_bounds[:-1]).max())
    G = (max_nodes + P - 1) // P

    deg_sorted_all = np.zeros((NC, G * P), np.int64)
    for c in range(NC):
        c0, c1 = int(node_bounds[c]), int(node_bounds[c + 1])
        deg_c = deg[c0:c1]
        order_c = np.argsort(-deg_c, kind="stable")
        ds = deg_c[order_c]
        deg_sorted_all[c, : ds.size] = ds
        cores.append({"c0": c0, "c1": c1, "order": order_c})

    Rg = deg_sorted_all[:, ::P].max(axis=0)[:G].astype(np.int64)
    Rg = Rg[Rg > 0]
    G_used = Rg.size
    goff = np.zeros(G_used + 1, np.int64)
    np.cumsum(Rg, out=goff[1:])

    # chunks of consecutive groups padded to a uniform per-chunk width R
    # (group widths are non-increasing, so padding stays small)
    chunks = []  # (ga, gb, R, off)
    gcol0 = np.zeros(G_used, np.int64)
    off = 0
    ga = 0
    while ga < G_used:
        R = int(Rg[ga])
        gb = ga + 1
        while gb < G_used:
            width = (gb - ga + 1) * R
            pad = width - int(goff[gb + 1] - goff[ga])
            if width > CHUNK or pad > 0.12 * width:
                break
            gb += 1
        chunks.append((ga, gb, R, off))
        gcol0[ga:gb] = off + (np.arange(ga, gb) - ga) * R
        off += (gb - ga) * R
        ga = gb
    F = int(off)

    for c in range(NC):
        info = cores[c]
        c0, c1, order_c = info["c0"], info["c1"], info["order"]
        n_nodes = c1 - c0
        ranks = np.empty(n_nodes, np.int64)
        ranks[order_c] = np.arange(n_nodes)

        e0, e1 = int(starts[c0]), int(starts[c1])
        l = dst_s[e0:e1] - c0
        r = ranks[l]
        gp = r >> 7
        pp = r & 127
        j = np.arange(e0, e1) - starts[dst_s[e0:e1]]
        col = gcol0[gp] + j
        flat = pp * F + col

        src_slot = np.full(P * F, -1, np.int64)
        src_slot[flat] = src_s[e0:e1]

        node_cols = np.full(P * G, -1, np.int64)
        rk = np.arange(n_nodes)
        keep = rk < G_used * P
        node_cols[(rk[keep] & 127) * G + (rk[keep] >> 7)] = c0 + order_c[keep]

        info.update(
            src_slot=src_slot,
            node_cols=node_cols,
            flat=flat,
            edge_ids=perm[e0:e1],
            n_nodes=n_nodes,
        )

    return {"G": G, "G_used": G_used, "Rg": Rg, "goff": goff, "F": F,
            "chunks": chunks, "gcol0": gcol0, "cores": cores}


def _build_table():
    import concourse.tile as tile
    from concourse import bacc, mybir
    from contextlib import ExitStack

    dt = mybir.dt
    MM = 512

    nc = bacc.Bacc("TRN2", target_bir_lowering=False, debug=False, num_devices=NC)
    xt_str = nc.dram_tensor("xt_str", [F_IN, NSTR_PAD], dt.float32, kind="ExternalInput").ap()
    wsrc = nc.dram_tensor("wsrc", [F_IN, C], dt.float32, kind="ExternalInput").ap()
    wsrcT = nc.dram_tensor("wsrcT", [C, F_IN], dt.float32, kind="ExternalInput").ap()
    att_s = nc.dram_tensor("att_s", [C, 1], dt.float32, kind="ExternalInput").ap()
    att_d = nc.dram_tensor("att_d", [C, 1], dt.float32, kind="ExternalInput").ap()
    g4sT = nc.dram_tensor("g4sT", [4, NSTR_PAD], dt.float32, kind="ExternalOutput").ap()

    with tile.TileContext(nc) as tc:
        with ExitStack() as ctx:
            const = ctx.enter_context(tc.tile_pool(name="const", bufs=1))
            psum = ctx.enter_context(tc.tile_pool(name="psum", bufs=4, space="PSUM"))
            xpool = ctx.enter_context(tc.tile_pool(name="xpool", bufs=1))

            w_sb = const.tile([F_IN, C], dt.float32)
            nc.sync.dma_start(w_sb[:], wsrc[:])
            wT_sb = const.tile([C, F_IN], dt.float32)
            nc.sync.dma_start(wT_sb[:], wsrcT[:])
            atts_sb = const.tile([C, 1], dt.float32)
            nc.sync.dma_start(atts_sb[:], att_s[:])
            attd_sb = const.tile([C, 1], dt.float32)
            nc.sync.dma_start(attd_sb[:], att_d[:])

            wcat = const.tile([F_IN, 4], dt.float32)
            asw_ps = psum.tile([F_IN, 1], dt.float32, tag="tmp_ps")
            nc.tensor.matmul(asw_ps[:], wT_sb[:], atts_sb[:], start=True, stop=True)
            nc.vector.tensor_copy(wcat[:, 0:1], asw_ps[:])
            adw_ps = psum.tile([F_IN, 1], dt.float32, tag="tmp_ps")
            nc.tensor.matmul(adw_ps[:], wT_sb[:], attd_sb[:], start=True, stop=True)
            nc.vector.tensor_copy(wcat[:, 1:2], adw_ps[:])
            nc.vector.tensor_copy(wcat[:, 2:4], w_sb[:])

            g4T_sb = xpool.tile([4, NSTR_PAD], dt.float32)
            XC = 1024
            nxc = (NSTR_PAD + XC - 1) // XC
            for kx in range(nxc):
                xa, xb = kx * XC, min((kx + 1) * XC, NSTR_PAD)
                xt_sb = const.tile([F_IN, XC], dt.float32, tag="xt", bufs=3)
                nc.sync.dma_start(xt_sb[:, : xb - xa], xt_str[:, xa:xb])
                for a in range(xa, xb, MM):
                    b = min(a + MM, xb)
                    m_ps = psum.tile([4, MM], dt.float32, tag="m_ps")
                    nc.tensor.matmul(
                        m_ps[:, : b - a], wcat[:], xt_sb[:, a - xa : b - xa],
                        start=True, stop=True,
                    )
                    if (a // MM) % 2 == 0:
                        nc.vector.tensor_copy(g4T_sb[:, a:b], m_ps[:, : b - a])
                    else:
                        nc.scalar.copy(g4T_sb[:, a:b], m_ps[:, : b - a])

            nc.sync.dma_start(g4sT[:], g4T_sb[:])

    nc.compile()
    return nc


def _build_edges(meta):
    import concourse.tile as tile
    from concourse import bacc, mybir
    from contextlib import ExitStack

    dt = mybir.dt
    AF = mybir.ActivationFunctionType
    ALU = mybir.AluOpType
    AX = mybir.AxisListType

    G = meta["G"]
    F = meta["F"]
    chunks = meta["chunks"]

    nc = bacc.Bacc("TRN2", target_bir_lowering=False, debug=False, num_devices=NC)
    wedge = nc.dram_tensor("wedge", [C, 1], dt.float32, kind="ExternalInput").ap()
    att_e = nc.dram_tensor("att_e", [C, 1], dt.float32, kind="ExternalInput").ap()
    bias_t = nc.dram_tensor("bias_t", [1, C], dt.float32, kind="ExternalInput").ap()
    pk = nc.dram_tensor("pk", [P, 4, F], dt.float32, kind="ExternalInput").ap()
    adstc = nc.dram_tensor("adstc", [P, G], dt.float32, kind="ExternalInput").ap()

    alpha_out = nc.dram_tensor("alpha_out", [P, F], dt.float32, kind="ExternalOutput").ap()
    outc = nc.dram_tensor("outc", [C, P, G], dt.float32, kind="ExternalOutput").ap()

    with tile.TileContext(nc) as tc:
        with ExitStack() as ctx:
            const = ctx.enter_context(tc.tile_pool(name="const", bufs=1))
            cols = ctx.enter_context(tc.tile_pool(name="cols", bufs=1))
            psum = ctx.enter_context(tc.tile_pool(name="psum", bufs=2, space="PSUM"))
            work = ctx.enter_context(tc.tile_pool(name="work", bufs=6))
            small = ctx.enter_context(tc.tile_pool(name="small", bufs=4))

            we_sb = const.tile([C, 1], dt.float32)
            nc.sync.dma_start(we_sb[:], wedge[:])
            ae_sb = const.tile([C, 1], dt.float32)
            nc.sync.dma_start(ae_sb[:], att_e[:])
            bias_sb = const.tile([1, C], dt.float32)
            nc.sync.dma_start(bias_sb[:], bias_t)
            ones_sb = const.tile([1, P], dt.float32)
            nc.vector.memset(ones_sb[:], 1.0)

            ce_ps = psum.tile([1, 1], dt.float32, tag="tmp_ps")
            nc.tensor.matmul(ce_ps[:], we_sb[:], ae_sb[:], start=True, stop=True)
            ce_sb = const.tile([1, 1], dt.float32)
            nc.vector.tensor_copy(ce_sb[:], ce_ps[:])
            cecol_ps = psum.tile([P, 1], dt.float32, tag="tmp_ps")
            nc.tensor.matmul(cecol_ps[:], ones_sb[:], ce_sb[:], start=True, stop=True)
            ce_col = const.tile([P, 1], dt.float32)
            nc.vector.tensor_copy(ce_col[:], cecol_ps[:])

            bias_cols = const.tile([P, C], dt.float32)
            b_ps = psum.tile([P, C], dt.float32, tag="tmp_ps")
            nc.tensor.matmul(b_ps[:], ones_sb[:], bias_sb[:], start=True, stop=True)
            nc.vector.tensor_copy(bias_cols[:], b_ps[:])

            adst_cols = cols.tile([P, G], dt.float32)
            nc.sync.dma_start(adst_cols[:], adstc[:])

            oc_sb = cols.tile([P, C, G], dt.float32)
            nc.vector.memset(oc_sb[:], 0.0)

            for (ga, gb, R, off) in chunks:
                gc = gb - ga
                cF = gc * R

                pk_t = work.tile([P, 4, cF], dt.float32, tag="pk")
                dmae = nc.sync if (ga % 2 == 0) else nc.scalar
                dmae.dma_start(pk_t[:], pk[:, :, off : off + cF])
                as_t = pk_t[:, 0, :]
                ea_t = pk_t[:, 1, :]
                h0_t = pk_t[:, 2, :]
                h1_t = pk_t[:, 3, :]

                # alpha = (ea*ce + a_src) + a_dst(broadcast)
                al_t = work.tile([P, cF], dt.float32, tag="al")
                nc.vector.scalar_tensor_tensor(
                    al_t[:], ea_t, ce_col[:, 0:1], as_t,
                    op0=ALU.mult, op1=ALU.add,
                )
                al3 = al_t[:].rearrange("p (g r) -> p g r", g=gc)
                adb = adst_cols[:, ga:gb].to_broadcast([P, gc, R])
                nc.vector.tensor_tensor(out=al3, in0=al3, in1=adb, op=ALU.add)

                # leaky relu + exp
                nc.scalar.activation(al_t[:], al_t[:], AF.Prelu, alpha=NEG_SLOPE)
                ex_t = work.tile([P, cF], dt.float32, tag="ex")
                nc.scalar.activation(ex_t[:], al_t[:], AF.Exp)
                ex3 = ex_t[:].rearrange("p (g r) -> p g r", g=gc)

                # segment sums (free-dim reduces)
                den_t = small.tile([P, gc], dt.float32, tag="den")
                nc.vector.tensor_reduce(den_t[:], ex3, axis=AX.X, op=ALU.add)
                nc.vector.tensor_scalar_add(den_t[:], den_t[:], 1e-16)
                rec_t = small.tile([P, gc], dt.float32, tag="rec")
                nc.vector.reciprocal(rec_t[:], den_t[:])
                reb = rec_t[:].to_broadcast([P, gc, R])

                # alpha_n out
                an_t = work.tile([P, cF], dt.float32, tag="an")
                an3 = an_t[:].rearrange("p (g r) -> p g r", g=gc)
                nc.vector.tensor_tensor(out=an3, in0=ex3, in1=reb, op=ALU.mult)
                dmao = nc.scalar if (ga % 2 == 0) else nc.sync
                dmao.dma_start(alpha_out[:, off : off + cF], an_t[:])

                # weighted aggregation
                nc.vector.tensor_tensor(out=h0_t, in0=ex_t[:], in1=h0_t, op=ALU.mult)
                nc.vector.tensor_tensor(out=h1_t, in0=ex_t[:], in1=h1_t, op=ALU.mult)
                h03 = h0_t.rearrange("p (g r) -> p g r", g=gc)
                h13 = h1_t.rearrange("p (g r) -> p g r", g=gc)
                ms0_t = small.tile([P, gc], dt.float32, tag="ms0")
                nc.vector.tensor_reduce(ms0_t[:], h03, axis=AX.X, op=ALU.add)
                ms1_t = small.tile([P, gc], dt.float32, tag="ms1")
                nc.vector.tensor_reduce(ms1_t[:], h13, axis=AX.X, op=ALU.add)
                nc.vector.tensor_tensor(
                    out=oc_sb[:, 0, ga:gb], in0=ms0_t[:], in1=rec_t[:], op=ALU.mult
                )
                nc.vector.tensor_tensor(
                    out=oc_sb[:, 1, ga:gb], in0=ms1_t[:], in1=rec_t[:], op=ALU.mult
                )

            for ch in range(C):
                nc.vector.tensor_scalar_add(
                    oc_sb[:, ch, :], oc_sb[:, ch, :], bias_cols[:, ch : ch + 1]
                )
                nc.sync.dma_start(outc[ch], oc_sb[:, ch, :])

    nc.compile()
    return nc


def _install_trace_hook():
    """Register the NTFF profiling hook (antenv.axon_hooks shim)."""
    import sys
    import types
    try:
        import antenv.axon_hooks  # noqa: F401
        return
    except ImportError:
        pass
    try:
        import trn_agent_boot.trn_boot as _tb
        hooks = types.ModuleType("antenv.axon_hooks")
        store = [None]
        hooks.set_axon_ntff_profile_hook = lambda h: store.__setitem__(0, h)
        hooks.get_axon_ntff_profile_hook = lambda: store[0]
        sys.modules["antenv.axon_hooks"] = hooks
        hooks.set_axon_ntff_profile_hook(
            _tb._ntff_profile_via_ctypes("/opt/axon/libaxon_pjrt.so")
        )
        import concourse.bass_utils as bu
        bu.upload_artifacts = lambda tmpdir: "local://skipped"
    except Exception:
        pass


def kernel(x, edge_index, edge_attr, W_src, W_edge, att_src, att_dst, att_edge, bias):
    from concourse.bass_utils import run_bass_kernel_spmd

    x = np.asarray(x, np.float32)
    W_src = np.asarray(W_src, np.float32)
    W_edge = np.asarray(W_edge, np.float32)
    att_src = np.asarray(att_src, np.float32)
    att_dst = np.asarray(att_dst, np.float32)
    att_edge = np.asarray(att_edge, np.float32)
    bias = np.asarray(bias, np.float32)
    edge_index = np.asarray(edge_index)
    ea = np.asarray(edge_attr, np.float32)[:, 0]

    meta = _prep(edge_index)
    G, F = meta["G"], meta["F"]
    trace = bool(int(os.environ.get("GAT_TRACE", "0")))
    if trace:
        _install_trace_hook()

    # ---- launch 1: node table ------------------------------------------
    nc1 = _build_table()
    in1 = []
    for c in range(NC):
        xt = np.zeros((F_IN, NSTR_PAD), np.float32)
        xt[:, :NSTR] = x[c * NSTR : (c + 1) * NSTR].T
        in1.append({
            "xt_str": xt, "wsrc": W_src,
            "wsrcT": np.ascontiguousarray(W_src.T),
            "att_s": att_src.reshape(C, 1), "att_d": att_dst.reshape(C, 1),
        })
    res1 = run_bass_kernel_spmd(nc1, in1, list(range(NC)), trace=trace)
    t1 = res1.exec_time_ns
    kernel.last_res1 = res1

    table = np.concatenate(
        [res1.results[c]["g4sT"][:, :NSTR].T for c in range(NC)], axis=0
    )  # [N, 4]

    # ---- host: pure-index gather into slab layout ----------------------
    nc2 = _build_edges(meta)
    in2 = []
    for c in range(NC):
        info = meta["cores"][c]
        ss = info["src_slot"]          # [P*F] node id or -1
        valid = ss >= 0
        gath = table[ss[valid]]        # [nnz, 4]
        asl = np.full(P * F, KILL, np.float32)
        eal = np.zeros(P * F, np.float32)
        h0l = np.zeros(P * F, np.float32)
        h1l = np.zeros(P * F, np.float32)
        asl[valid] = gath[:, 0]
        h0l[valid] = gath[:, 2]
        h1l[valid] = gath[:, 3]
        eal[info["flat"]] = ea[info["edge_ids"]]
        pkl = np.stack(
            [asl.reshape(P, F), eal.reshape(P, F),
             h0l.reshape(P, F), h1l.reshape(P, F)], axis=1
        )
        ncl = info["node_cols"]
        adl = np.where(ncl >= 0, table[np.maximum(ncl, 0), 1], 0.0).astype(np.float32)
        in2.append({
            "wedge": W_edge.reshape(C, 1), "att_e": att_edge.reshape(C, 1),
            "bias_t": bias.reshape(1, C),
            "pk": pkl,
            "adstc": adl.reshape(P, G),
        })
    res2 = run_bass_kernel_spmd(nc2, in2, list(range(NC)), trace=trace)
    t2 = res2.exec_time_ns
    kernel.last_res2 = res2
    kernel.last_exec_time_ns = (t1 or 0) + (t2 or 0) if (t1 or t2) else None

    # ---- host: unshard outputs -----------------------------------------
    out = np.broadcast_to(bias.reshape(1, C), (N, C)).astype(np.float32).copy()
    alpha = np.zeros(E, np.float32)
    for c in range(NC):
        info = meta["cores"][c]
        r = res2.results[c]
        alpha[info["edge_ids"]] = r["alpha_out"].reshape(-1)[info["flat"]]
        oc = r["outc"]
        n_nodes = info["n_nodes"]
        rk = np.arange(n_nodes)
        vals = oc[:, rk & 127, rk >> 7]
        out[info["c0"] + info["order"]] = vals.T
    return out, alpha[:, None]


kernel.last_exec_time_ns = None


# revision 36
# speedup vs baseline: 1.0472x; 1.0472x over previous
"""GAT message-passing kernel for Trainium2, 8 NeuronCores (SPMD).

Two SPMD launches:
1. "table": each core computes G4.T = [W@att_src, W@att_dst, W].T @ x.T for
   its 1/8 stripe of nodes (Wcat stationary on the PE, x.T streamed).
   The host concatenates the stripes into the [N,4] node table and performs
   the pure-index gather of per-edge (a_src, h0, h1) into the slab layout
   (data movement only; all math stays on device).
2. "edges": per core, its edge slab (counting-sorted by dst, degree-grouped
   padded CSR on 128 partitions, uniform group width per chunk) streams
   through fused scalar_tensor_tensor / Prelu / Exp; segment softmax and
   the weighted aggregation are strided free-dim reduces; per-node values
   broadcast via stride-0 access patterns.

Each core owns a contiguous dst-node range (~E/8 edges), so segment
reductions are fully core-local; no collectives are needed.
"""

import os
import numpy as np

N = 100000
E = 6400000
F_IN = 128
C = 2
NC = 8
P = 128
NEG_SLOPE = 0.2
NSTR = N // NC          # 12500 nodes per x-stripe
TILES = (NSTR + P - 1) // P
NSTR_PAD = TILES * P    # 12544
KILL = -1.0e5
CHUNK = 512


def _prep(edge_index):
    src = np.ascontiguousarray(edge_index[0]).astype(np.int64)
    dst = np.ascontiguousarray(edge_index[1]).astype(np.int64)

    perm = np.argsort(dst, kind="stable")
    dst_s = dst[perm]
    src_s = src[perm]

    deg = np.bincount(dst, minlength=N).astype(np.int64)
    starts = np.zeros(N + 1, np.int64)
    np.cumsum(deg, out=starts[1:])

    targets = (np.arange(1, NC) * E) // NC
    nb = np.searchsorted(starts, targets).astype(np.int64)
    node_bounds = np.concatenate([[0], nb, [N]])

    cores = []
    max_nodes = int((node_bounds[1:] - node_bounds[:-1]).max())
    G = (max_nodes + P - 1) // P

    deg_sorted_all = np.zeros((NC, G * P), np.int64)
    for c in range(NC):
        c0, c1 = int(node_bounds[c]), int(node_bounds[c + 1])
        deg_c = deg[c0:c1]
        order_c = np.argsort(-deg_c, kind="stable")
        ds = deg_c[order_c]
        deg_sorted_all[c, : ds.size] = ds
        cores.append({"c0": c0, "c1": c1, "order": order_c})

    Rg = deg_sorted_all[:, ::P].max(axis=0)[:G].astype(np.int64)
    Rg = Rg[Rg > 0]
    G_used = Rg.size
    goff = np.zeros(G_used + 1, np.int64)
    np.cumsum(Rg, out=goff[1:])

    # chunks of consecutive groups padded to a uniform per-chunk width R
    # (group widths are non-increasing, so padding stays small)
    chunks = []  # (ga, gb, R, off)
    gcol0 = np.zeros(G_used, np.int64)
    off = 0
    ga = 0
    while ga < G_used:
        R = int(Rg[ga])
        gb = ga + 1
        while gb < G_used:
            width = (gb - ga + 1) * R
            pad = width - int(goff[gb + 1] - goff[ga])
            if width > CHUNK or pad > 0.12 * width:
                break
            gb += 1
        chunks.append((ga, gb, R, off))
        gcol0[ga:gb] = off + (np.arange(ga, gb) - ga) * R
        off += (gb - ga) * R
        ga = gb
    F = int(off)

    for c in range(NC):
        info = cores[c]
        c0, c1, order_c = info["c0"], info["c1"], info["order"]
        n_nodes = c1 - c0
        ranks = np.empty(n_nodes, np.int64)
        ranks[order_c] = np.arange(n_nodes)

        e0, e1 = int(starts[c0]), int(starts[c1])
        l = dst_s[e0:e1] - c0
        r = ranks[l]
        gp = r >> 7
        pp = r & 127
        j = np.arange(e0, e1) - starts[dst_s[e0:e1]]
        col = gcol0[gp] + j
        flat = pp * F + col

        src_slot = np.full(P * F, -1, np.int64)
        src_slot[flat] = src_s[e0:e1]

        node_cols = np.full(P * G, -1, np.int64)
        rk = np.arange(n_nodes)
        keep = rk < G_used * P
        node_cols[(rk[keep] & 127) * G + (rk[keep] >> 7)] = c0 + order_c[keep]

        info.update(
            src_slot=src_slot,
            node_cols=node_cols,
            flat=flat,
            edge_ids=perm[e0:e1],
            n_nodes=n_nodes,
        )

    return {"G": G, "G_used": G_used, "Rg": Rg, "goff": goff, "F": F,
            "chunks": chunks, "gcol0": gcol0, "cores": cores}


def _build_table():
    import concourse.tile as tile
    from concourse import bacc, mybir
    from contextlib import ExitStack

    dt = mybir.dt
    MM = 512

    nc = bacc.Bacc("TRN2", target_bir_lowering=False, debug=False, num_devices=NC)
    xt_str = nc.dram_tensor("xt_str", [F_IN, NSTR_PAD], dt.float32, kind="ExternalInput").ap()
    wsrc = nc.dram_tensor("wsrc", [F_IN, C], dt.float32, kind="ExternalInput").ap()
    wsrcT = nc.dram_tensor("wsrcT", [C, F_IN], dt.float32, kind="ExternalInput").ap()
    att_s = nc.dram_tensor("att_s", [C, 1], dt.float32, kind="ExternalInput").ap()
    att_d = nc.dram_tensor("att_d", [C, 1], dt.float32, kind="ExternalInput").ap()
    g4sT = nc.dram_tensor("g4sT", [4, NSTR_PAD], dt.float32, kind="ExternalOutput").ap()

    with tile.TileContext(nc) as tc:
        with ExitStack() as ctx:
            const = ctx.enter_context(tc.tile_pool(name="const", bufs=1))
            psum = ctx.enter_context(tc.tile_pool(name="psum", bufs=4, space="PSUM"))
            xpool = ctx.enter_context(tc.tile_pool(name="xpool", bufs=1))

            w_sb = const.tile([F_IN, C], dt.float32)
            nc.sync.dma_start(w_sb[:], wsrc[:])
            wT_sb = const.tile([C, F_IN], dt.float32)
            nc.sync.dma_start(wT_sb[:], wsrcT[:])
            atts_sb = const.tile([C, 1], dt.float32)
            nc.sync.dma_start(atts_sb[:], att_s[:])
            attd_sb = const.tile([C, 1], dt.float32)
            nc.sync.dma_start(attd_sb[:], att_d[:])

            wcat = const.tile([F_IN, 4], dt.float32)
            asw_ps = psum.tile([F_IN, 1], dt.float32, tag="tmp_ps")
            nc.tensor.matmul(asw_ps[:], wT_sb[:], atts_sb[:], start=True, stop=True)
            nc.vector.tensor_copy(wcat[:, 0:1], asw_ps[:])
            adw_ps = psum.tile([F_IN, 1], dt.float32, tag="tmp_ps")
            nc.tensor.matmul(adw_ps[:], wT_sb[:], attd_sb[:], start=True, stop=True)
            nc.vector.tensor_copy(wcat[:, 1:2], adw_ps[:])
            nc.vector.tensor_copy(wcat[:, 2:4], w_sb[:])

            g4T_sb = xpool.tile([4, NSTR_PAD], dt.float32)
            XC = 2048
            nxc = (NSTR_PAD + XC - 1) // XC
            for kx in range(nxc):
                xa, xb = kx * XC, min((kx + 1) * XC, NSTR_PAD)
                xt_sb = const.tile([F_IN, XC], dt.float32, tag="xt", bufs=3)
                nc.sync.dma_start(xt_sb[:, : xb - xa], xt_str[:, xa:xb])
                for a in range(xa, xb, MM):
                    b = min(a + MM, xb)
                    m_ps = psum.tile([4, MM], dt.float32, tag="m_ps")
                    nc.tensor.matmul(
                        m_ps[:, : b - a], wcat[:], xt_sb[:, a - xa : b - xa],
                        start=True, stop=True,
                    )
                    if (a // MM) % 2 == 0:
                        nc.vector.tensor_copy(g4T_sb[:, a:b], m_ps[:, : b - a])
                    else:
                        nc.scalar.copy(g4T_sb[:, a:b], m_ps[:, : b - a])

            nc.sync.dma_start(g4sT[:], g4T_sb[:])

    nc.compile()
    return nc


def _build_edges(meta):
    import concourse.tile as tile
    from concourse import bacc, mybir
    from contextlib import ExitStack

    dt = mybir.dt
    AF = mybir.ActivationFunctionType
    ALU = mybir.AluOpType
    AX = mybir.AxisListType

    G = meta["G"]
    F = meta["F"]
    chunks = meta["chunks"]

    nc = bacc.Bacc("TRN2", target_bir_lowering=False, debug=False, num_devices=NC)
    wedge = nc.dram_tensor("wedge", [C, 1], dt.float32, kind="ExternalInput").ap()
    att_e = nc.dram_tensor("att_e", [C, 1], dt.float32, kind="ExternalInput").ap()
    bias_t = nc.dram_tensor("bias_t", [1, C], dt.float32, kind="ExternalInput").ap()
    pk = nc.dram_tensor("pk", [P, 4, F], dt.float32, kind="ExternalInput").ap()
    adstc = nc.dram_tensor("adstc", [P, G], dt.float32, kind="ExternalInput").ap()

    alpha_out = nc.dram_tensor("alpha_out", [P, F], dt.float32, kind="ExternalOutput").ap()
    outc = nc.dram_tensor("outc", [C, P, G], dt.float32, kind="ExternalOutput").ap()

    with tile.TileContext(nc) as tc:
        with ExitStack() as ctx:
            const = ctx.enter_context(tc.tile_pool(name="const", bufs=1))
            cols = ctx.enter_context(tc.tile_pool(name="cols", bufs=1))
            psum = ctx.enter_context(tc.tile_pool(name="psum", bufs=2, space="PSUM"))
            work = ctx.enter_context(tc.tile_pool(name="work", bufs=6))
            small = ctx.enter_context(tc.tile_pool(name="small", bufs=4))

            we_sb = const.tile([C, 1], dt.float32)
            nc.sync.dma_start(we_sb[:], wedge[:])
            ae_sb = const.tile([C, 1], dt.float32)
            nc.sync.dma_start(ae_sb[:], att_e[:])
            bias_sb = const.tile([1, C], dt.float32)
            nc.sync.dma_start(bias_sb[:], bias_t)
            ones_sb = const.tile([1, P], dt.float32)
            nc.vector.memset(ones_sb[:], 1.0)

            ce_ps = psum.tile([1, 1], dt.float32, tag="tmp_ps")
            nc.tensor.matmul(ce_ps[:], we_sb[:], ae_sb[:], start=True, stop=True)
            ce_sb = const.tile([1, 1], dt.float32)
            nc.vector.tensor_copy(ce_sb[:], ce_ps[:])
            cecol_ps = psum.tile([P, 1], dt.float32, tag="tmp_ps")
            nc.tensor.matmul(cecol_ps[:], ones_sb[:], ce_sb[:], start=True, stop=True)
            ce_col = const.tile([P, 1], dt.float32)
            nc.vector.tensor_copy(ce_col[:], cecol_ps[:])

            bias_cols = const.tile([P, C], dt.float32)
            b_ps = psum.tile([P, C], dt.float32, tag="tmp_ps")
            nc.tensor.matmul(b_ps[:], ones_sb[:], bias_sb[:], start=True, stop=True)
            nc.vector.tensor_copy(bias_cols[:], b_ps[:])

            adst_cols = cols.tile([P, G], dt.float32)
            nc.sync.dma_start(adst_cols[:], adstc[:])

            oc_sb = cols.tile([P, C, G], dt.float32)
            nc.vector.memset(oc_sb[:], 0.0)

            for (ga, gb, R, off) in chunks:
                gc = gb - ga
                cF = gc * R

                pk_t = work.tile([P, 4, cF], dt.float32, tag="pk")
                nc.sync.dma_start(pk_t[:], pk[:, :, off : off + cF])
                as_t = pk_t[:, 0, :]
                ea_t = pk_t[:, 1, :]
                h0_t = pk_t[:, 2, :]
                h1_t = pk_t[:, 3, :]

                # alpha = (ea*ce + a_src) + a_dst(broadcast)
                al_t = work.tile([P, cF], dt.float32, tag="al")
                nc.vector.scalar_tensor_tensor(
                    al_t[:], ea_t, ce_col[:, 0:1], as_t,
                    op0=ALU.mult, op1=ALU.add,
                )
                al3 = al_t[:].rearrange("p (g r) -> p g r", g=gc)
                adb = adst_cols[:, ga:gb].to_broadcast([P, gc, R])
                nc.vector.tensor_tensor(out=al3, in0=al3, in1=adb, op=ALU.add)

                # leaky relu + exp
                nc.scalar.activation(al_t[:], al_t[:], AF.Prelu, alpha=NEG_SLOPE)
                ex_t = work.tile([P, cF], dt.float32, tag="ex")
                nc.scalar.activation(ex_t[:], al_t[:], AF.Exp)
                ex3 = ex_t[:].rearrange("p (g r) -> p g r", g=gc)

                # segment sums (free-dim reduces)
                den_t = small.tile([P, gc], dt.float32, tag="den")
                nc.vector.tensor_reduce(den_t[:], ex3, axis=AX.X, op=ALU.add)
                nc.vector.tensor_scalar_add(den_t[:], den_t[:], 1e-16)
                rec_t = small.tile([P, gc], dt.float32, tag="rec")
                nc.vector.reciprocal(rec_t[:], den_t[:])
                reb = rec_t[:].to_broadcast([P, gc, R])

                # alpha_n out
                an_t = work.tile([P, cF], dt.float32, tag="an")
                an3 = an_t[:].rearrange("p (g r) -> p g r", g=gc)
                nc.vector.tensor_tensor(out=an3, in0=ex3, in1=reb, op=ALU.mult)
                nc.sync.dma_start(alpha_out[:, off : off + cF], an_t[:])

                # weighted aggregation
                nc.vector.tensor_tensor(out=h0_t, in0=ex_t[:], in1=h0_t, op=ALU.mult)
                nc.vector.tensor_tensor(out=h1_t, in0=ex_t[:], in1=h1_t, op=ALU.mult)
                h03 = h0_t.rearrange("p (g r) -> p g r", g=gc)
                h13 = h1_t.rearrange("p (g r) -> p g r", g=gc)
                ms0_t = small.tile([P, gc], dt.float32, tag="ms0")
                nc.vector.tensor_reduce(ms0_t[:], h03, axis=AX.X, op=ALU.add)
                ms1_t = small.tile([P, gc], dt.float32, tag="ms1")
                nc.vector.tensor_reduce(ms1_t[:], h13, axis=AX.X, op=ALU.add)
                nc.vector.tensor_tensor(
                    out=oc_sb[:, 0, ga:gb], in0=ms0_t[:], in1=rec_t[:], op=ALU.mult
                )
                nc.vector.tensor_tensor(
                    out=oc_sb[:, 1, ga:gb], in0=ms1_t[:], in1=rec_t[:], op=ALU.mult
                )

            for ch in range(C):
                nc.vector.tensor_scalar_add(
                    oc_sb[:, ch, :], oc_sb[:, ch, :], bias_cols[:, ch : ch + 1]
                )
                nc.sync.dma_start(outc[ch], oc_sb[:, ch, :])

    nc.compile()
    return nc


def _install_trace_hook():
    """Register the NTFF profiling hook (antenv.axon_hooks shim)."""
    import sys
    import types
    try:
        import antenv.axon_hooks  # noqa: F401
        return
    except ImportError:
        pass
    try:
        import trn_agent_boot.trn_boot as _tb
        hooks = types.ModuleType("antenv.axon_hooks")
        store = [None]
        hooks.set_axon_ntff_profile_hook = lambda h: store.__setitem__(0, h)
        hooks.get_axon_ntff_profile_hook = lambda: store[0]
        sys.modules["antenv.axon_hooks"] = hooks
        hooks.set_axon_ntff_profile_hook(
            _tb._ntff_profile_via_ctypes("/opt/axon/libaxon_pjrt.so")
        )
        import concourse.bass_utils as bu
        bu.upload_artifacts = lambda tmpdir: "local://skipped"
    except Exception:
        pass


def kernel(x, edge_index, edge_attr, W_src, W_edge, att_src, att_dst, att_edge, bias):
    from concourse.bass_utils import run_bass_kernel_spmd

    x = np.asarray(x, np.float32)
    W_src = np.asarray(W_src, np.float32)
    W_edge = np.asarray(W_edge, np.float32)
    att_src = np.asarray(att_src, np.float32)
    att_dst = np.asarray(att_dst, np.float32)
    att_edge = np.asarray(att_edge, np.float32)
    bias = np.asarray(bias, np.float32)
    edge_index = np.asarray(edge_index)
    ea = np.asarray(edge_attr, np.float32)[:, 0]

    meta = _prep(edge_index)
    G, F = meta["G"], meta["F"]
    trace = bool(int(os.environ.get("GAT_TRACE", "0")))
    if trace:
        _install_trace_hook()

    # ---- launch 1: node table ------------------------------------------
    nc1 = _build_table()
    in1 = []
    for c in range(NC):
        xt = np.zeros((F_IN, NSTR_PAD), np.float32)
        xt[:, :NSTR] = x[c * NSTR : (c + 1) * NSTR].T
        in1.append({
            "xt_str": xt, "wsrc": W_src,
            "wsrcT": np.ascontiguousarray(W_src.T),
            "att_s": att_src.reshape(C, 1), "att_d": att_dst.reshape(C, 1),
        })
    res1 = run_bass_kernel_spmd(nc1, in1, list(range(NC)), trace=trace)
    t1 = res1.exec_time_ns
    kernel.last_res1 = res1

    table = np.concatenate(
        [res1.results[c]["g4sT"][:, :NSTR].T for c in range(NC)], axis=0
    )  # [N, 4]

    # ---- host: pure-index gather into slab layout ----------------------
    nc2 = _build_edges(meta)
    in2 = []
    for c in range(NC):
        info = meta["cores"][c]
        ss = info["src_slot"]          # [P*F] node id or -1
        valid = ss >= 0
        gath = table[ss[valid]]        # [nnz, 4]
        asl = np.full(P * F, KILL, np.float32)
        eal = np.zeros(P * F, np.float32)
        h0l = np.zeros(P * F, np.float32)
        h1l = np.zeros(P * F, np.float32)
        asl[valid] = gath[:, 0]
        h0l[valid] = gath[:, 2]
        h1l[valid] = gath[:, 3]
        eal[info["flat"]] = ea[info["edge_ids"]]
        pkl = np.stack(
            [asl.reshape(P, F), eal.reshape(P, F),
             h0l.reshape(P, F), h1l.reshape(P, F)], axis=1
        )
        ncl = info["node_cols"]
        adl = np.where(ncl >= 0, table[np.maximum(ncl, 0), 1], 0.0).astype(np.float32)
        in2.append({
            "wedge": W_edge.reshape(C, 1), "att_e": att_edge.reshape(C, 1),
            "bias_t": bias.reshape(1, C),
            "pk": pkl,
            "adstc": adl.reshape(P, G),
        })
    res2 = run_bass_kernel_spmd(nc2, in2, list(range(NC)), trace=trace)
    t2 = res2.exec_time_ns
    kernel.last_res2 = res2
    kernel.last_exec_time_ns = (t1 or 0) + (t2 or 0) if (t1 or t2) else None

    # ---- host: unshard outputs -----------------------------------------
    out = np.broadcast_to(bias.reshape(1, C), (N, C)).astype(np.float32).copy()
    alpha = np.zeros(E, np.float32)
    for c in range(NC):
        info = meta["cores"][c]
        r = res2.results[c]
        alpha[info["edge_ids"]] = r["alpha_out"].reshape(-1)[info["flat"]]
        oc = r["outc"]
        n_nodes = info["n_nodes"]
        rk = np.arange(n_nodes)
        vals = oc[:, rk & 127, rk >> 7]
        out[info["c0"] + info["order"]] = vals.T
    return out, alpha[:, None]


kernel.last_exec_time_ns = None


# revision 37
# speedup vs baseline: 1.0861x; 1.0371x over previous
"""GAT message-passing kernel for Trainium2, 8 NeuronCores (SPMD).

Two SPMD launches:
1. "table": each core computes G4.T = [W@att_src, W@att_dst, W].T @ x.T for
   its 1/8 stripe of nodes (Wcat stationary on the PE, x.T streamed).
   The host concatenates the stripes into the [N,4] node table and performs
   the pure-index gather of per-edge (a_src, h0, h1) into the slab layout
   (data movement only; all math stays on device).
2. "edges": per core, its edge slab (counting-sorted by dst, degree-grouped
   padded CSR on 128 partitions, uniform group width per chunk) streams
   through fused scalar_tensor_tensor / Prelu / Exp; segment softmax and
   the weighted aggregation are strided free-dim reduces; per-node values
   broadcast via stride-0 access patterns.

Each core owns a contiguous dst-node range (~E/8 edges), so segment
reductions are fully core-local; no collectives are needed.
"""

import os
import numpy as np

N = 100000
E = 6400000
F_IN = 128
C = 2
NC = 8
P = 128
NEG_SLOPE = 0.2
NSTR = N // NC          # 12500 nodes per x-stripe
TILES = (NSTR + P - 1) // P
NSTR_PAD = TILES * P    # 12544
KILL = -1.0e5
CHUNK = 768


def _prep(edge_index):
    src = np.ascontiguousarray(edge_index[0]).astype(np.int64)
    dst = np.ascontiguousarray(edge_index[1]).astype(np.int64)

    perm = np.argsort(dst, kind="stable")
    dst_s = dst[perm]
    src_s = src[perm]

    deg = np.bincount(dst, minlength=N).astype(np.int64)
    starts = np.zeros(N + 1, np.int64)
    np.cumsum(deg, out=starts[1:])

    targets = (np.arange(1, NC) * E) // NC
    nb = np.searchsorted(starts, targets).astype(np.int64)
    node_bounds = np.concatenate([[0], nb, [N]])

    cores = []
    max_nodes = int((node_bounds[1:] - node_bounds[:-1]).max())
    G = (max_nodes + P - 1) // P

    deg_sorted_all = np.zeros((NC, G * P), np.int64)
    for c in range(NC):
        c0, c1 = int(node_bounds[c]), int(node_bounds[c + 1])
        deg_c = deg[c0:c1]
        order_c = np.argsort(-deg_c, kind="stable")
        ds = deg_c[order_c]
        deg_sorted_all[c, : ds.size] = ds
        cores.append({"c0": c0, "c1": c1, "order": order_c})

    Rg = deg_sorted_all[:, ::P].max(axis=0)[:G].astype(np.int64)
    Rg = Rg[Rg > 0]
    G_used = Rg.size
    goff = np.zeros(G_used + 1, np.int64)
    np.cumsum(Rg, out=goff[1:])

    # chunks of consecutive groups padded to a uniform per-chunk width R
    # (group widths are non-increasing, so padding stays small)
    chunks = []  # (ga, gb, R, off)
    gcol0 = np.zeros(G_used, np.int64)
    off = 0
    ga = 0
    while ga < G_used:
        R = int(Rg[ga])
        gb = ga + 1
        while gb < G_used:
            width = (gb - ga + 1) * R
            pad = width - int(goff[gb + 1] - goff[ga])
            if width > CHUNK or pad > 0.12 * width:
                break
            gb += 1
        chunks.append((ga, gb, R, off))
        gcol0[ga:gb] = off + (np.arange(ga, gb) - ga) * R
        off += (gb - ga) * R
        ga = gb
    F = int(off)

    for c in range(NC):
        info = cores[c]
        c0, c1, order_c = info["c0"], info["c1"], info["order"]
        n_nodes = c1 - c0
        ranks = np.empty(n_nodes, np.int64)
        ranks[order_c] = np.arange(n_nodes)

        e0, e1 = int(starts[c0]), int(starts[c1])
        l = dst_s[e0:e1] - c0
        r = ranks[l]
        gp = r >> 7
        pp = r & 127
        j = np.arange(e0, e1) - starts[dst_s[e0:e1]]
        col = gcol0[gp] + j
        flat = pp * F + col

        src_slot = np.full(P * F, -1, np.int64)
        src_slot[flat] = src_s[e0:e1]

        node_cols = np.full(P * G, -1, np.int64)
        rk = np.arange(n_nodes)
        keep = rk < G_used * P
        node_cols[(rk[keep] & 127) * G + (rk[keep] >> 7)] = c0 + order_c[keep]

        info.update(
            src_slot=src_slot,
            node_cols=node_cols,
            flat=flat,
            edge_ids=perm[e0:e1],
            n_nodes=n_nodes,
        )

    return {"G": G, "G_used": G_used, "Rg": Rg, "goff": goff, "F": F,
            "chunks": chunks, "gcol0": gcol0, "cores": cores}


def _build_table():
    import concourse.tile as tile
    from concourse import bacc, mybir
    from contextlib import ExitStack

    dt = mybir.dt
    MM = 512

    nc = bacc.Bacc("TRN2", target_bir_lowering=False, debug=False, num_devices=NC)
    xt_str = nc.dram_tensor("xt_str", [F_IN, NSTR_PAD], dt.float32, kind="ExternalInput").ap()
    wsrc = nc.dram_tensor("wsrc", [F_IN, C], dt.float32, kind="ExternalInput").ap()
    wsrcT = nc.dram_tensor("wsrcT", [C, F_IN], dt.float32, kind="ExternalInput").ap()
    att_s = nc.dram_tensor("att_s", [C, 1], dt.float32, kind="ExternalInput").ap()
    att_d = nc.dram_tensor("att_d", [C, 1], dt.float32, kind="ExternalInput").ap()
    g4sT = nc.dram_tensor("g4sT", [4, NSTR_PAD], dt.float32, kind="ExternalOutput").ap()

    with tile.TileContext(nc) as tc:
        with ExitStack() as ctx:
            const = ctx.enter_context(tc.tile_pool(name="const", bufs=1))
            psum = ctx.enter_context(tc.tile_pool(name="psum", bufs=4, space="PSUM"))
            xpool = ctx.enter_context(tc.tile_pool(name="xpool", bufs=1))

            w_sb = const.tile([F_IN, C], dt.float32)
            nc.sync.dma_start(w_sb[:], wsrc[:])
            wT_sb = const.tile([C, F_IN], dt.float32)
            nc.sync.dma_start(wT_sb[:], wsrcT[:])
            atts_sb = const.tile([C, 1], dt.float32)
            nc.sync.dma_start(atts_sb[:], att_s[:])
            attd_sb = const.tile([C, 1], dt.float32)
            nc.sync.dma_start(attd_sb[:], att_d[:])

            wcat = const.tile([F_IN, 4], dt.float32)
            asw_ps = psum.tile([F_IN, 1], dt.float32, tag="tmp_ps")
            nc.tensor.matmul(asw_ps[:], wT_sb[:], atts_sb[:], start=True, stop=True)
            nc.vector.tensor_copy(wcat[:, 0:1], asw_ps[:])
            adw_ps = psum.tile([F_IN, 1], dt.float32, tag="tmp_ps")
            nc.tensor.matmul(adw_ps[:], wT_sb[:], attd_sb[:], start=True, stop=True)
            nc.vector.tensor_copy(wcat[:, 1:2], adw_ps[:])
            nc.vector.tensor_copy(wcat[:, 2:4], w_sb[:])

            g4T_sb = xpool.tile([4, NSTR_PAD], dt.float32)
            XC = 2048
            nxc = (NSTR_PAD + XC - 1) // XC
            for kx in range(nxc):
                xa, xb = kx * XC, min((kx + 1) * XC, NSTR_PAD)
                xt_sb = const.tile([F_IN, XC], dt.float32, tag="xt", bufs=3)
                nc.sync.dma_start(xt_sb[:, : xb - xa], xt_str[:, xa:xb])
                for a in range(xa, xb, MM):
                    b = min(a + MM, xb)
                    m_ps = psum.tile([4, MM], dt.float32, tag="m_ps")
                    nc.tensor.matmul(
                        m_ps[:, : b - a], wcat[:], xt_sb[:, a - xa : b - xa],
                        start=True, stop=True,
                    )
                    if (a // MM) % 2 == 0:
                        nc.vector.tensor_copy(g4T_sb[:, a:b], m_ps[:, : b - a])
                    else:
                        nc.scalar.copy(g4T_sb[:, a:b], m_ps[:, : b - a])

            nc.sync.dma_start(g4sT[:], g4T_sb[:])

    nc.compile()
    return nc


def _build_edges(meta):
    import concourse.tile as tile
    from concourse import bacc, mybir
    from contextlib import ExitStack

    dt = mybir.dt
    AF = mybir.ActivationFunctionType
    ALU = mybir.AluOpType
    AX = mybir.AxisListType

    G = meta["G"]
    F = meta["F"]
    chunks = meta["chunks"]

    nc = bacc.Bacc("TRN2", target_bir_lowering=False, debug=False, num_devices=NC)
    wedge = nc.dram_tensor("wedge", [C, 1], dt.float32, kind="ExternalInput").ap()
    att_e = nc.dram_tensor("att_e", [C, 1], dt.float32, kind="ExternalInput").ap()
    bias_t = nc.dram_tensor("bias_t", [1, C], dt.float32, kind="ExternalInput").ap()
    pk = nc.dram_tensor("pk", [P, 4, F], dt.float32, kind="ExternalInput").ap()
    adstc = nc.dram_tensor("adstc", [P, G], dt.float32, kind="ExternalInput").ap()

    alpha_out = nc.dram_tensor("alpha_out", [P, F], dt.float32, kind="ExternalOutput").ap()
    outc = nc.dram_tensor("outc", [C, P, G], dt.float32, kind="ExternalOutput").ap()

    with tile.TileContext(nc) as tc:
        with ExitStack() as ctx:
            const = ctx.enter_context(tc.tile_pool(name="const", bufs=1))
            cols = ctx.enter_context(tc.tile_pool(name="cols", bufs=1))
            psum = ctx.enter_context(tc.tile_pool(name="psum", bufs=2, space="PSUM"))
            work = ctx.enter_context(tc.tile_pool(name="work", bufs=6))
            small = ctx.enter_context(tc.tile_pool(name="small", bufs=4))

            we_sb = const.tile([C, 1], dt.float32)
            nc.sync.dma_start(we_sb[:], wedge[:])
            ae_sb = const.tile([C, 1], dt.float32)
            nc.sync.dma_start(ae_sb[:], att_e[:])
            bias_sb = const.tile([1, C], dt.float32)
            nc.sync.dma_start(bias_sb[:], bias_t)
            ones_sb = const.tile([1, P], dt.float32)
            nc.vector.memset(ones_sb[:], 1.0)

            ce_ps = psum.tile([1, 1], dt.float32, tag="tmp_ps")
            nc.tensor.matmul(ce_ps[:], we_sb[:], ae_sb[:], start=True, stop=True)
            ce_sb = const.tile([1, 1], dt.float32)
            nc.vector.tensor_copy(ce_sb[:], ce_ps[:])
            cecol_ps = psum.tile([P, 1], dt.float32, tag="tmp_ps")
            nc.tensor.matmul(cecol_ps[:], ones_sb[:], ce_sb[:], start=True, stop=True)
            ce_col = const.tile([P, 1], dt.float32)
            nc.vector.tensor_copy(ce_col[:], cecol_ps[:])

            bias_cols = const.tile([P, C], dt.float32)
            b_ps = psum.tile([P, C], dt.float32, tag="tmp_ps")
            nc.tensor.matmul(b_ps[:], ones_sb[:], bias_sb[:], start=True, stop=True)
            nc.vector.tensor_copy(bias_cols[:], b_ps[:])

            adst_cols = cols.tile([P, G], dt.float32)
            nc.sync.dma_start(adst_cols[:], adstc[:])

            oc_sb = cols.tile([P, C, G], dt.float32)
            nc.vector.memset(oc_sb[:], 0.0)

            for (ga, gb, R, off) in chunks:
                gc = gb - ga
                cF = gc * R

                pk_t = work.tile([P, 4, cF], dt.float32, tag="pk")
                nc.sync.dma_start(pk_t[:], pk[:, :, off : off + cF])
                as_t = pk_t[:, 0, :]
                ea_t = pk_t[:, 1, :]
                h0_t = pk_t[:, 2, :]
                h1_t = pk_t[:, 3, :]

                # alpha = (ea*ce + a_src) + a_dst(broadcast)
                al_t = work.tile([P, cF], dt.float32, tag="al")
                nc.vector.scalar_tensor_tensor(
                    al_t[:], ea_t, ce_col[:, 0:1], as_t,
                    op0=ALU.mult, op1=ALU.add,
                )
                al3 = al_t[:].rearrange("p (g r) -> p g r", g=gc)
                adb = adst_cols[:, ga:gb].to_broadcast([P, gc, R])
                nc.vector.tensor_tensor(out=al3, in0=al3, in1=adb, op=ALU.add)

                # leaky relu + exp
                nc.scalar.activation(al_t[:], al_t[:], AF.Prelu, alpha=NEG_SLOPE)
                ex_t = work.tile([P, cF], dt.float32, tag="ex")
                nc.scalar.activation(ex_t[:], al_t[:], AF.Exp)
                ex3 = ex_t[:].rearrange("p (g r) -> p g r", g=gc)

                # segment sums (free-dim reduces)
                den_t = small.tile([P, gc], dt.float32, tag="den")
                nc.vector.tensor_reduce(den_t[:], ex3, axis=AX.X, op=ALU.add)
                nc.vector.tensor_scalar_add(den_t[:], den_t[:], 1e-16)
                rec_t = small.tile([P, gc], dt.float32, tag="rec")
                nc.vector.reciprocal(rec_t[:], den_t[:])
                reb = rec_t[:].to_broadcast([P, gc, R])

                # alpha_n out
                an_t = work.tile([P, cF], dt.float32, tag="an")
                an3 = an_t[:].rearrange("p (g r) -> p g r", g=gc)
                nc.vector.tensor_tensor(out=an3, in0=ex3, in1=reb, op=ALU.mult)
                nc.sync.dma_start(alpha_out[:, off : off + cF], an_t[:])

                # weighted aggregation
                nc.vector.tensor_tensor(out=h0_t, in0=ex_t[:], in1=h0_t, op=ALU.mult)
                nc.vector.tensor_tensor(out=h1_t, in0=ex_t[:], in1=h1_t, op=ALU.mult)
                h03 = h0_t.rearrange("p (g r) -> p g r", g=gc)
                h13 = h1_t.rearrange("p (g r) -> p g r", g=gc)
                ms0_t = small.tile([P, gc], dt.float32, tag="ms0")
                nc.vector.tensor_reduce(ms0_t[:], h03, axis=AX.X, op=ALU.add)
                ms1_t = small.tile([P, gc], dt.float32, tag="ms1")
                nc.vector.tensor_reduce(ms1_t[:], h13, axis=AX.X, op=ALU.add)
                nc.vector.tensor_tensor(
                    out=oc_sb[:, 0, ga:gb], in0=ms0_t[:], in1=rec_t[:], op=ALU.mult
                )
                nc.vector.tensor_tensor(
                    out=oc_sb[:, 1, ga:gb], in0=ms1_t[:], in1=rec_t[:], op=ALU.mult
                )

            for ch in range(C):
                nc.vector.tensor_scalar_add(
                    oc_sb[:, ch, :], oc_sb[:, ch, :], bias_cols[:, ch : ch + 1]
                )
                nc.sync.dma_start(outc[ch], oc_sb[:, ch, :])

    nc.compile()
    return nc


def _install_trace_hook():
    """Register the NTFF profiling hook (antenv.axon_hooks shim)."""
    import sys
    import types
    try:
        import antenv.axon_hooks  # noqa: F401
        return
    except ImportError:
        pass
    try:
        import trn_agent_boot.trn_boot as _tb
        hooks = types.ModuleType("antenv.axon_hooks")
        store = [None]
        hooks.set_axon_ntff_profile_hook = lambda h: store.__setitem__(0, h)
        hooks.get_axon_ntff_profile_hook = lambda: store[0]
        sys.modules["antenv.axon_hooks"] = hooks
        hooks.set_axon_ntff_profile_hook(
            _tb._ntff_profile_via_ctypes("/opt/axon/libaxon_pjrt.so")
        )
        import concourse.bass_utils as bu
        bu.upload_artifacts = lambda tmpdir: "local://skipped"
    except Exception:
        pass


def kernel(x, edge_index, edge_attr, W_src, W_edge, att_src, att_dst, att_edge, bias):
    from concourse.bass_utils import run_bass_kernel_spmd

    x = np.asarray(x, np.float32)
    W_src = np.asarray(W_src, np.float32)
    W_edge = np.asarray(W_edge, np.float32)
    att_src = np.asarray(att_src, np.float32)
    att_dst = np.asarray(att_dst, np.float32)
    att_edge = np.asarray(att_edge, np.float32)
    bias = np.asarray(bias, np.float32)
    edge_index = np.asarray(edge_index)
    ea = np.asarray(edge_attr, np.float32)[:, 0]

    meta = _prep(edge_index)
    G, F = meta["G"], meta["F"]
    trace = bool(int(os.environ.get("GAT_TRACE", "0")))
    if trace:
        _install_trace_hook()

    # ---- launch 1: node table ------------------------------------------
    nc1 = _build_table()
    in1 = []
    for c in range(NC):
        xt = np.zeros((F_IN, NSTR_PAD), np.float32)
        xt[:, :NSTR] = x[c * NSTR : (c + 1) * NSTR].T
        in1.append({
            "xt_str": xt, "wsrc": W_src,
            "wsrcT": np.ascontiguousarray(W_src.T),
            "att_s": att_src.reshape(C, 1), "att_d": att_dst.reshape(C, 1),
        })
    res1 = run_bass_kernel_spmd(nc1, in1, list(range(NC)), trace=trace)
    t1 = res1.exec_time_ns
    kernel.last_res1 = res1

    table = np.concatenate(
        [res1.results[c]["g4sT"][:, :NSTR].T for c in range(NC)], axis=0
    )  # [N, 4]

    # ---- host: pure-index gather into slab layout ----------------------
    nc2 = _build_edges(meta)
    in2 = []
    for c in range(NC):
        info = meta["cores"][c]
        ss = info["src_slot"]          # [P*F] node id or -1
        valid = ss >= 0
        gath = table[ss[valid]]        # [nnz, 4]
        asl = np.full(P * F, KILL, np.float32)
        eal = np.zeros(P * F, np.float32)
        h0l = np.zeros(P * F, np.float32)
        h1l = np.zeros(P * F, np.float32)
        asl[valid] = gath[:, 0]
        h0l[valid] = gath[:, 2]
        h1l[valid] = gath[:, 3]
        eal[info["flat"]] = ea[info["edge_ids"]]
        pkl = np.stack(
            [asl.reshape(P, F), eal.reshape(P, F),
             h0l.reshape(P, F), h1l.reshape(P, F)], axis=1
        )
        ncl = info["node_cols"]
        adl = np.where(ncl >= 0, table[np.maximum(ncl, 0), 1], 0.0).astype(np.float32)
        in2.append({
            "wedge": W_edge.reshape(C, 1), "att_e": att_edge.reshape(C, 1),
            "bias_t": bias.reshape(1, C),
            "pk": pkl,
            "adstc": adl.reshape(P, G),
        })
    res2 = run_bass_kernel_spmd(nc2, in2, list(range(NC)), trace=trace)
    t2 = res2.exec_time_ns
    kernel.last_res2 = res2
    kernel.last_exec_time_ns = (t1 or 0) + (t2 or 0) if (t1 or t2) else None

    # ---- host: unshard outputs -----------------------------------------
    out = np.broadcast_to(bias.reshape(1, C), (N, C)).astype(np.float32).copy()
    alpha = np.zeros(E, np.float32)
    for c in range(NC):
        info = meta["cores"][c]
        r = res2.results[c]
        alpha[info["edge_ids"]] = r["alpha_out"].reshape(-1)[info["flat"]]
        oc = r["outc"]
        n_nodes = info["n_nodes"]
        rk = np.arange(n_nodes)
        vals = oc[:, rk & 127, rk >> 7]
        out[info["c0"] + info["order"]] = vals.T
    return out, alpha[:, None]


kernel.last_exec_time_ns = None
